# revision 36
# baseline (speedup 1.0000x reference)
"""Trainium2 Bass kernel for the segment-reduce masked-CE loss (nn_NewLoss).

Reference math (N=64, C=46, P=2048, MP=256):
    assignment[n, p] = 1 + (p * MP) // P  (contiguous segments of 8 frames)
    pooled[n, q, c]  = mean over the 8 frames of segment q of input[n, c, :]
    loss = -sum_{n,q} lab_mask[n,q] * log_softmax(pooled)[n, q, target[n,q]]

Sharding: data-parallel over batch n across 8 cores (8 items per core);
each core returns a partial-loss scalar, summed on the host.

Per-core layout: the 368 local (item, channel) rows are packed into 3 slots
of 128 partitions (zero-padded to 384) with q=0..255 along the free dim:
    slot tile X_s[u, p],  row r = 128*s + u = 46*item + c
Pipeline per slot: DMA -> window-8 reduce (DVE) -> exp (ACT) -> PE item-sum
matmuls (sumexp + picked-class via an on-device one-hot built from a PE
broadcast of targets), then a short log/mask/reduce epilogue.
"""

import numpy as np

import concourse.bacc as bacc
import concourse.bass as bass
import concourse.tile as tile
from concourse import mybir
from concourse.bass_utils import run_bass_kernel_spmd

F32 = mybir.dt.float32
BF16 = mybir.dt.bfloat16

N, C, P, MP = 64, 46, 2048, 256
NCORES = 8
NLOC = N // NCORES            # 8 batch items per core
ROWS = NLOC * C               # 368 (item, channel) rows per core
SLOTS = (ROWS + 127) // 128   # 3 partition slots
W = P // MP                   # 8-frame pooling window

# aux column layout (fp32, [128, AUXW]):
#   [0:256)    rows 0..7: mask8 (lab_mask per (item, q))
#   [256:259)  c_col per slot: c(128s+u) = (128s+u) % 46 for valid rows, -1 pad
#   [259:260)  ones
OFF_MSK = 0
OFF_CCOL = MP
OFF_ONES = OFF_CCOL + SLOTS
AUXW = OFF_ONES + 1

# selb column layout (bf16, [128, SELW]) -- PE operands; 0/1 selectors are
# exact in bf16, so matmuls run single-pass:
#   [0:24)     isel:  isel[u, 8s+i] = 1 iff row 128s+u belongs to item i
#   [24:792)   ohp: per-slot masked one-hot, ohp[u, 256s+q] =
#              -1/8 if c(128s+u) == target[item(128s+u), q] else 0
OFF_B_ISEL = 0
OFF_B_OHP = NLOC * SLOTS
SELW = OFF_B_OHP + SLOTS * MP


def _build_nc():
    nc = bacc.Bacc("TRN2", target_bir_lowering=False)

    x_d = nc.dram_tensor("x", [128, SLOTS * P], BF16, kind="ExternalInput")
    aux_d = nc.dram_tensor("aux", [128, AUXW], F32, kind="ExternalInput")
    selb_d = nc.dram_tensor("selb", [128, SELW], BF16, kind="ExternalInput")
    loss_d = nc.dram_tensor("loss", [NLOC, 2], F32, kind="ExternalOutput")

    with tile.TileContext(nc) as tc:
        with (
            tc.tile_pool(name="xin", bufs=1) as xin,
            tc.tile_pool(name="pp", bufs=1) as pp,
            tc.tile_pool(name="psum", bufs=2, space="PSUM") as psum,
            tc.tile_pool(name="acc", bufs=1, space="PSUM") as accp,
            tc.tile_pool(name="small", bufs=1) as small,
        ):
            # x first on both HWDGE rings (sync + scalar) so the pool reduces
            # start as early as possible; aux/selb are only needed late, so
            # they queue after x.  The first chunk is small (512 cols) to cut
            # the DMA latency before the first reduce; chunk boundaries land
            # on 8-col windows so each reduce covers whole q ranges.
            chunk_cols = {
                0: [(0, 512), (512, 1280), (1280, 2048)],
                1: [(0, 1024), (1024, 2048)],
                2: [(0, 1024), (1024, 2048)],
            }
            # All x chunks go on the sync ring: HWDGE FIFO per ring means
            # they complete in consumption order, keeping the DVE reduce
            # pipeline gapless (a single DMA already spans all 16 SDMA
            # engines, so one ring gets full bandwidth).  Constants ride
            # the scalar ring in parallel.
            xs = []
            for s in range(SLOTS):
                xt = xin.tile([128, P], BF16, tag=f"x{s}")
                for c0, c1 in chunk_cols[s]:
                    nc.sync.dma_start(
                        out=xt[:, c0:c1],
                        in_=x_d[:, s * P + c0 : s * P + c1],
                    )
                xs.append(xt)

            aux_t = small.tile([128, AUXW], F32)
            nc.scalar.dma_start(out=aux_t[:], in_=aux_d[:])
            selb_t = small.tile([128, SELW], BF16)
            nc.scalar.dma_start(out=selb_t[:], in_=selb_d[:])
            msk8 = aux_t[0:NLOC, OFF_MSK : OFF_MSK + MP]
            ones8 = aux_t[0:NLOC, OFF_ONES : OFF_ONES + 1]

            s8_t = accp.tile([NLOC, MP], F32, tag="S8")
            px8_t = accp.tile([NLOC, MP], F32, tag="PX8")
            for s in range(SLOTS):
                isel_s = selb_t[:, OFF_B_ISEL + NLOC * s : OFF_B_ISEL + NLOC * (s + 1)]
                ohp_s = selb_t[:, OFF_B_OHP + MP * s : OFF_B_OHP + MP * (s + 1)]

                p_t = pp.tile([128, MP], F32, tag=f"p{s}")
                for c0, c1 in chunk_cols[s]:
                    nc.vector.reduce_sum(
                        out=p_t[:, c0 // W : c1 // W],
                        in_=xs[s][:, c0:c1].rearrange("u (q w) -> u q w", w=W),
                        axis=mybir.AxisListType.X,
                    )
                # sumexp: S8[i, q] += sum_u isel[u, i] * exp(pooled[u, q] / 8)
                xe_t = pp.tile([128, MP], BF16, tag=f"xe{s}")
                nc.scalar.activation(
                    out=xe_t[:],
                    in_=p_t[:],
                    func=mybir.ActivationFunctionType.Exp,
                    scale=1.0 / W,
                )
                nc.tensor.matmul(
                    out=s8_t[:],
                    lhsT=isel_s,
                    rhs=xe_t[:],
                    start=(s == 0),
                    stop=(s == SLOTS - 1),
                )
                # picked: M = ohp * pooled, summed per item by the PE.  The
                # last slot's multiply runs on DVE (free after its reduces)
                # to keep the tail off the slower GPSIMD path.
                m_t = pp.tile([128, MP], BF16, tag=f"m{s}")
                m_eng = nc.vector if s == SLOTS - 1 else nc.gpsimd
                m_eng.tensor_tensor(m_t[:], ohp_s, p_t[:], mybir.AluOpType.mult)
                nc.tensor.matmul(
                    out=px8_t[:],
                    lhsT=isel_s,
                    rhs=m_t[:],
                    start=(s == 0),
                    stop=(s == SLOTS - 1),
                )

            # The lab mask is folded into ohp on the host, so px8 is already
            # masked: reduce it as soon as it lands (overlaps the Ln table
            # load).  loss = sum_q msk*ln(S8) + sum_q px8, summed on host.
            cv_t = small.tile([NLOC, 2], F32)
            nc.vector.reduce_sum(
                out=cv_t[:, 1:2], in_=px8_t[:], axis=mybir.AxisListType.X
            )
            lse_t = small.tile([NLOC, MP], F32)
            nc.scalar.activation(
                out=lse_t[:], in_=s8_t[:], func=mybir.ActivationFunctionType.Ln
            )
            z_t = small.tile([NLOC, MP], F32)
            nc.vector.tensor_tensor(z_t[:], lse_t[:], msk8, mybir.AluOpType.mult)
            nc.vector.reduce_sum(
                out=cv_t[:, 0:1], in_=z_t[:], axis=mybir.AxisListType.X
            )
            nc.sync.dma_start(out=loss_d[:], in_=cv_t[:])

    nc.finalize()
    return nc


_NC = None


def _get_nc():
    global _NC
    if _NC is None:
        _NC = _build_nc()
    return _NC


def _make_aux():
    import ml_dtypes

    aux = np.zeros((128, AUXW), dtype=np.float32)
    selb = np.zeros((128, SELW), dtype=ml_dtypes.bfloat16)
    rows = np.arange(SLOTS * 128)
    item = np.minimum(rows // C, NLOC - 1)
    valid = rows < ROWS
    isel = np.zeros((SLOTS * 128, NLOC), dtype=np.float32)
    isel[valid, item[valid]] = 1.0
    isel = isel.reshape(SLOTS, 128, NLOC)
    for s in range(SLOTS):
        selb[:, OFF_B_ISEL + NLOC * s : OFF_B_ISEL + NLOC * (s + 1)] = isel[s]
    aux[:, OFF_ONES] = 1.0
    return aux, selb


def make_in_maps(input, target, lab_mask):
    import ml_dtypes

    inp = np.asarray(input)
    tgt = np.asarray(target)
    msk = np.asarray(lab_mask)
    aux_base, selb_base = _make_aux()
    in_maps = []
    for c in range(NCORES):
        xl = np.asarray(inp[c * NLOC : (c + 1) * NLOC], dtype=ml_dtypes.bfloat16)
        xl = xl.reshape(ROWS, P)
        xp = np.zeros((SLOTS * 128, P), dtype=ml_dtypes.bfloat16)
        xp[:ROWS] = xl
        xd = np.ascontiguousarray(
            xp.reshape(SLOTS, 128, P).transpose(1, 0, 2).reshape(128, SLOTS * P)
        )
        aux = aux_base.copy()
        aux[0:NLOC, OFF_MSK : OFF_MSK + MP] = msk[c * NLOC : (c + 1) * NLOC].astype(
            np.float32
        )
        selb = selb_base.copy()
        tl = tgt[c * NLOC : (c + 1) * NLOC]  # [8, 256] int
        rows = np.arange(SLOTS * 128)
        item = np.minimum(rows // C, NLOC - 1)
        cval = rows % C
        valid = rows < ROWS
        ml = msk[c * NLOC : (c + 1) * NLOC].astype(np.float32)  # [8, 256]
        ohp = (tl[item, :] == cval[:, None]) & valid[:, None]
        ohp = ohp.astype(np.float32) * (-1.0 / W) * ml[item, :]
        ohp = ohp.reshape(SLOTS, 128, MP)
        for s in range(SLOTS):
            selb[:, OFF_B_OHP + MP * s : OFF_B_OHP + MP * (s + 1)] = ohp[s].astype(
                ml_dtypes.bfloat16
            )
        in_maps.append({"x": xd, "aux": aux, "selb": selb})
    return in_maps


def kernel(input, target, assignment, lab_mask, _trace=False):
    in_maps = make_in_maps(input, target, lab_mask)
    nc = _get_nc()
    res = run_bass_kernel_spmd(nc, in_maps, core_ids=list(range(NCORES)), trace=_trace)
    total = np.float64(0.0)
    for r in res.results:
        total += np.float64(r["loss"].sum())
    out = np.array(total, dtype=np.float32)
    if _trace:
        return out, res
    return out


# revision 37
# speedup vs baseline: 1.0105x; 1.0105x over previous
"""Trainium2 Bass kernel for the segment-reduce masked-CE loss (nn_NewLoss).

Reference math (N=64, C=46, P=2048, MP=256):
    assignment[n, p] = 1 + (p * MP) // P  (contiguous segments of 8 frames)
    pooled[n, q, c]  = mean over the 8 frames of segment q of input[n, c, :]
    loss = -sum_{n,q} lab_mask[n,q] * log_softmax(pooled)[n, q, target[n,q]]

Sharding: data-parallel over batch n across 8 cores (8 items per core);
each core returns a partial-loss scalar, summed on the host.

Per-core layout: the 368 local (item, channel) rows are packed into 3 slots
of 128 partitions (zero-padded to 384) with q=0..255 along the free dim:
    slot tile X_s[u, p],  row r = 128*s + u = 46*item + c
Pipeline per slot: DMA -> window-8 reduce (DVE) -> exp (ACT) -> PE item-sum
matmuls (sumexp + picked-class via an on-device one-hot built from a PE
broadcast of targets), then a short log/mask/reduce epilogue.
"""

import numpy as np

import concourse.bacc as bacc
import concourse.bass as bass
import concourse.tile as tile
from concourse import mybir
from concourse.bass_utils import run_bass_kernel_spmd

F32 = mybir.dt.float32
BF16 = mybir.dt.bfloat16

N, C, P, MP = 64, 46, 2048, 256
NCORES = 8
NLOC = N // NCORES            # 8 batch items per core
ROWS = NLOC * C               # 368 (item, channel) rows per core
SLOTS = (ROWS + 127) // 128   # 3 partition slots
W = P // MP                   # 8-frame pooling window

# aux column layout (fp32, [128, AUXW]):
#   [0:256)    rows 0..7: mask8 (lab_mask per (item, q))
#   [256:259)  c_col per slot: c(128s+u) = (128s+u) % 46 for valid rows, -1 pad
#   [259:260)  ones
OFF_MSK = 0
OFF_CCOL = MP
OFF_ONES = OFF_CCOL + SLOTS
AUXW = OFF_ONES + 1

# selb column layout (bf16, [128, SELW]) -- PE operands; 0/1 selectors are
# exact in bf16, so matmuls run single-pass:
#   [0:24)     isel:  isel[u, 8s+i] = 1 iff row 128s+u belongs to item i
#   [24:792)   ohp: per-slot masked one-hot, ohp[u, 256s+q] =
#              -1/8 if c(128s+u) == target[item(128s+u), q] else 0
OFF_B_ISEL = 0
OFF_B_OHP = NLOC * SLOTS
SELW = OFF_B_OHP + SLOTS * MP


def _build_nc():
    nc = bacc.Bacc("TRN2", target_bir_lowering=False)

    x_d = nc.dram_tensor("x", [128, SLOTS * P], BF16, kind="ExternalInput")
    aux_d = nc.dram_tensor("aux", [128, AUXW], F32, kind="ExternalInput")
    selb_d = nc.dram_tensor("selb", [128, SELW], BF16, kind="ExternalInput")
    loss_d = nc.dram_tensor("loss", [NLOC, 2], F32, kind="ExternalOutput")

    with tile.TileContext(nc) as tc:
        with (
            tc.tile_pool(name="xin", bufs=1) as xin,
            tc.tile_pool(name="pp", bufs=1) as pp,
            tc.tile_pool(name="psum", bufs=2, space="PSUM") as psum,
            tc.tile_pool(name="acc", bufs=1, space="PSUM") as accp,
            tc.tile_pool(name="small", bufs=1) as small,
        ):
            # x first on both HWDGE rings (sync + scalar) so the pool reduces
            # start as early as possible; aux/selb are only needed late, so
            # they queue after x.  The first chunk is small (512 cols) to cut
            # the DMA latency before the first reduce; chunk boundaries land
            # on 8-col windows so each reduce covers whole q ranges.
            chunk_cols = {
                0: [(0, 512), (512, 1280), (1280, 2048)],
                1: [(0, 1024), (1024, 2048)],
                2: [(0, 1024), (1024, 2048)],
            }
            # All x chunks go on the sync ring: HWDGE FIFO per ring means
            # they complete in consumption order, keeping the DVE reduce
            # pipeline gapless (a single DMA already spans all 16 SDMA
            # engines, so one ring gets full bandwidth).  Constants ride
            # the scalar ring in parallel.
            xs = []
            for s in range(SLOTS):
                xt = xin.tile([128, P], BF16, tag=f"x{s}")
                for c0, c1 in chunk_cols[s]:
                    nc.sync.dma_start(
                        out=xt[:, c0:c1],
                        in_=x_d[:, s * P + c0 : s * P + c1],
                    )
                xs.append(xt)

            aux_t = small.tile([128, AUXW], F32)
            nc.scalar.dma_start(out=aux_t[:], in_=aux_d[:])
            selb_t = small.tile([128, SELW], BF16)
            nc.scalar.dma_start(out=selb_t[:], in_=selb_d[:])
            msk8 = aux_t[0:NLOC, OFF_MSK : OFF_MSK + MP]
            ones8 = aux_t[0:NLOC, OFF_ONES : OFF_ONES + 1]

            s8_t = accp.tile([NLOC, MP], F32, tag="S8")
            px8_t = accp.tile([NLOC, MP], F32, tag="PX8")
            for s in range(SLOTS):
                isel_s = selb_t[:, OFF_B_ISEL + NLOC * s : OFF_B_ISEL + NLOC * (s + 1)]
                ohp_s = selb_t[:, OFF_B_OHP + MP * s : OFF_B_OHP + MP * (s + 1)]

                # Window-8 pool in two stages: a bf16 tensor_tensor fold
                # (8->4, runs at DVE 2x_1P mode: 16-bit + inner step 1) then
                # a 1x-mode reduce over the remaining 4 -- halves the reads
                # the 1x reduce has to stream.
                p_t = pp.tile([128, MP], F32, tag=f"p{s}")
                f_t = pp.tile([128, P // 2], BF16, tag=f"f{s}")
                for c0, c1 in chunk_cols[s]:
                    xv3 = xs[s][:, c0:c1].rearrange("u (q w) -> u q w", w=W)
                    fv3 = f_t[:, c0 // 2 : c1 // 2].rearrange(
                        "u (q w) -> u q w", w=W // 2
                    )
                    nc.vector.tensor_tensor(
                        fv3, xv3[:, :, 0 : W // 2], xv3[:, :, W // 2 : W],
                        mybir.AluOpType.add,
                    )
                    nc.vector.reduce_sum(
                        out=p_t[:, c0 // W : c1 // W],
                        in_=fv3,
                        axis=mybir.AxisListType.X,
                    )
                # sumexp: S8[i, q] += sum_u isel[u, i] * exp(pooled[u, q] / 8)
                xe_t = pp.tile([128, MP], BF16, tag=f"xe{s}")
                nc.scalar.activation(
                    out=xe_t[:],
                    in_=p_t[:],
                    func=mybir.ActivationFunctionType.Exp,
                    scale=1.0 / W,
                )
                nc.tensor.matmul(
                    out=s8_t[:],
                    lhsT=isel_s,
                    rhs=xe_t[:],
                    start=(s == 0),
                    stop=(s == SLOTS - 1),
                )
                # picked: M = ohp * pooled, summed per item by the PE.  The
                # last slot's multiply runs on DVE (free after its reduces)
                # to keep the tail off the slower GPSIMD path.
                m_t = pp.tile([128, MP], BF16, tag=f"m{s}")
                m_eng = nc.vector if s == SLOTS - 1 else nc.gpsimd
                m_eng.tensor_tensor(m_t[:], ohp_s, p_t[:], mybir.AluOpType.mult)
                nc.tensor.matmul(
                    out=px8_t[:],
                    lhsT=isel_s,
                    rhs=m_t[:],
                    start=(s == 0),
                    stop=(s == SLOTS - 1),
                )

            # The lab mask is folded into ohp on the host, so px8 is already
            # masked: reduce it as soon as it lands (overlaps the Ln table
            # load).  loss = sum_q msk*ln(S8) + sum_q px8, summed on host.
            cv_t = small.tile([NLOC, 2], F32)
            nc.vector.reduce_sum(
                out=cv_t[:, 1:2], in_=px8_t[:], axis=mybir.AxisListType.X
            )
            lse_t = small.tile([NLOC, MP], F32)
            nc.scalar.activation(
                out=lse_t[:], in_=s8_t[:], func=mybir.ActivationFunctionType.Ln
            )
            z_t = small.tile([NLOC, MP], F32)
            nc.vector.tensor_tensor(z_t[:], lse_t[:], msk8, mybir.AluOpType.mult)
            nc.vector.reduce_sum(
                out=cv_t[:, 0:1], in_=z_t[:], axis=mybir.AxisListType.X
            )
            nc.sync.dma_start(out=loss_d[:], in_=cv_t[:])

    nc.finalize()
    return nc


_NC = None


def _get_nc():
    global _NC
    if _NC is None:
        _NC = _build_nc()
    return _NC


def _make_aux():
    import ml_dtypes

    aux = np.zeros((128, AUXW), dtype=np.float32)
    selb = np.zeros((128, SELW), dtype=ml_dtypes.bfloat16)
    rows = np.arange(SLOTS * 128)
    item = np.minimum(rows // C, NLOC - 1)
    valid = rows < ROWS
    isel = np.zeros((SLOTS * 128, NLOC), dtype=np.float32)
    isel[valid, item[valid]] = 1.0
    isel = isel.reshape(SLOTS, 128, NLOC)
    for s in range(SLOTS):
        selb[:, OFF_B_ISEL + NLOC * s : OFF_B_ISEL + NLOC * (s + 1)] = isel[s]
    aux[:, OFF_ONES] = 1.0
    return aux, selb


def make_in_maps(input, target, lab_mask):
    import ml_dtypes

    inp = np.asarray(input)
    tgt = np.asarray(target)
    msk = np.asarray(lab_mask)
    aux_base, selb_base = _make_aux()
    in_maps = []
    for c in range(NCORES):
        xl = np.asarray(inp[c * NLOC : (c + 1) * NLOC], dtype=ml_dtypes.bfloat16)
        xl = xl.reshape(ROWS, P)
        xp = np.zeros((SLOTS * 128, P), dtype=ml_dtypes.bfloat16)
        xp[:ROWS] = xl
        xd = np.ascontiguousarray(
            xp.reshape(SLOTS, 128, P).transpose(1, 0, 2).reshape(128, SLOTS * P)
        )
        aux = aux_base.copy()
        aux[0:NLOC, OFF_MSK : OFF_MSK + MP] = msk[c * NLOC : (c + 1) * NLOC].astype(
            np.float32
        )
        selb = selb_base.copy()
        tl = tgt[c * NLOC : (c + 1) * NLOC]  # [8, 256] int
        rows = np.arange(SLOTS * 128)
        item = np.minimum(rows // C, NLOC - 1)
        cval = rows % C
        valid = rows < ROWS
        ml = msk[c * NLOC : (c + 1) * NLOC].astype(np.float32)  # [8, 256]
        ohp = (tl[item, :] == cval[:, None]) & valid[:, None]
        ohp = ohp.astype(np.float32) * (-1.0 / W) * ml[item, :]
        ohp = ohp.reshape(SLOTS, 128, MP)
        for s in range(SLOTS):
            selb[:, OFF_B_OHP + MP * s : OFF_B_OHP + MP * (s + 1)] = ohp[s].astype(
                ml_dtypes.bfloat16
            )
        in_maps.append({"x": xd, "aux": aux, "selb": selb})
    return in_maps


def kernel(input, target, assignment, lab_mask, _trace=False):
    in_maps = make_in_maps(input, target, lab_mask)
    nc = _get_nc()
    res = run_bass_kernel_spmd(nc, in_maps, core_ids=list(range(NCORES)), trace=_trace)
    total = np.float64(0.0)
    for r in res.results:
        total += np.float64(r["loss"].sum())
    out = np.array(total, dtype=np.float32)
    if _trace:
        return out, res
    return out
